# revision 14
# baseline (speedup 1.0000x reference)
"""Trainium2 Bass kernel for nn_DecoderLSTM.

Data-parallel over batch: B=256 split across 8 NeuronCores (b=32 each),
all LSTM/conv weights replicated. Each core runs the full decoder on its
batch slice; host gathers per-core outputs.

Device layout (per core):
  - Activations stored transposed: feature-on-partition, (t, b) on free dim.
  - LSTM gates computed as gates.T = Whh @ h.T + Wih @ x.T + b, accumulated
    in PSUM. Gate m-tiles (128 rows of 4H) reordered to (i0,i1,f0,f1,o0,o1,g0,g1)
    so sigmoid/tanh each cover contiguous tile ranges.
  - PSUM gate tile: [128, 4 banks x 512] per block of 4 timesteps;
    bank = m-pair, col = (m&1)*256 + d*128 + slot*32 + b.
    Backward direction runs its sequence reversed with slot = 3-j so its
    per-block input projections are written with ascending APs.
  - Input projections (xg) + biases are pre-accumulated into the same PSUM
    banks by blocked GEMMs (N=128), so activations read gates straight from
    PSUM with no separate add.
  - Weights in bf16 (FWL weight-load path), cell state in fp32.
"""

import sys

sys.path.insert(0, "/opt/trn_rl_repo")

import numpy as np
import ml_dtypes

import concourse.bass as bass
from concourse import bacc
import concourse.mybir as mybir
import concourse.tile as tile
from concourse.bass_utils import run_bass_kernel_spmd

F32 = mybir.dt.float32
BF16 = mybir.dt.bfloat16
AF = mybir.ActivationFunctionType
NPBF = ml_dtypes.bfloat16

B, H, CNN, PD = 256, 256, 64, 96
NCORES = 8
b = B // NCORES  # 32
P = 128

# (stage key, T, enc_h index, layer group, n_layers, din of layer0, residual,
#  head conv key, upsample (n_src, repeat) or None)
STAGES = [
    dict(key="s0", T=8, hidx=0, lg="l00", nl=1, din0=2, res=False, head="c00", up=None),
    dict(key="s1", T=12, hidx=1, lg="l01", nl=1, din0=2 * H, res=True, head="c01", up=(3, 4)),
    dict(key="s3", T=36, hidx=3, lg="l03", nl=2, din0=2 * H, res=True, head="c03", up=(6, 6)),
    dict(key="s4", T=96, hidx=4, lg="l04", nl=2, din0=2 * H, res=True, head="c04", up=(32, 3)),
    dict(key="s5", T=96, hidx=5, lg="l05", nl=2, din0=3, res=False, head="c02", up=None),
]

HEADS = {  # conv chains (Cout, Cin, k); taken from reference params structure
    "c00": [(CNN, 2 * H, 3), (1, CNN, 5)],
    "c01": [(CNN, 2 * H, 3), (1, CNN, 5)],
    "c03": [(2 * CNN, 2 * H, 3), (CNN, 2 * CNN, 5), (1, CNN, 7)],
    "c04": [(H, 2 * H, 5), (1, H, 7)],
    "c02": [(H, 2 * H, 5), (1, H, 7)],
}

# gate row-block permutation: torch (i,f,g,o) -> ours (i,f,o,g)
def _gate_perm():
    idx = np.arange(4 * H).reshape(4, H)  # i,f,g,o
    return np.concatenate([idx[0], idx[1], idx[3], idx[2]])  # i,f,o,g


GP = _gate_perm()

# y segments (cols of the [1, YN] output, (t, b) packed)
YSEG = {"out0": (0, 8 * b), "out1": (256, 12 * b), "out2": (640, 96 * b),
        "out3": (3712, 36 * b), "out4": (4864, 96 * b)}
YN = 7936


def _to_np(x):
    return np.asarray(x)


class _Cursor:
    def __init__(self):
        self.n = 0

    def take(self, k):
        o = self.n
        self.n += k
        return o


def _pack_weights(params):
    """Build wpack (bf16 [128, WN]) + wpack32 (fp32 [128, WN32]) + offsets."""
    off = {}
    cur = _Cursor()
    cur32 = _Cursor()
    blocks = []      # (col_off, np [128, cols] bf16)
    blocks32 = []    # (col_off, np [rows, cols] f32, row0)

    def put(name, arr):  # arr [128, cols] bf16
        o = cur.take(arr.shape[1])
        off[name] = o
        blocks.append((o, arr))

    def put_row0(name, arr):  # arr [rows<=128, cols]
        o = cur32.take(arr.shape[1])
        off[name] = o
        blocks32.append((o, arr))

    # LSTM layers
    for st in STAGES:
        for li in range(st["nl"]):
            p = params[st["lg"]][li]
            Wih = _to_np(p["Wih"]).astype(np.float32)  # [2, 4H, din]
            Whh = _to_np(p["Whh"]).astype(np.float32)  # [2, 4H, H]
            bb_ = _to_np(p["b"]).astype(np.float32)    # [2, 4H]
            din = Wih.shape[2]
            name = f"{st['key']}_l{li}"
            # whh: [128, (d,k,m,128)]
            whh_cols = np.zeros((P, 2 * 2 * 8 * P), dtype=NPBF)
            for d in range(2):
                Wt = Whh[d][GP].T  # [H, 4H]
                for k in range(2):
                    for m in range(8):
                        tileW = Wt[k * P:(k + 1) * P, m * P:(m + 1) * P]
                        c0 = ((d * 2 + k) * 8 + m) * P
                        whh_cols[:, c0:c0 + P] = tileW.astype(NPBF)
            put(name + "_whh", whh_cols)
            if din <= P - 1:
                # single padded k-tile, bias folded in as row `din`
                wih_cols = np.zeros((P, 2 * 8 * P), dtype=NPBF)
                for d in range(2):
                    Wt = Wih[d][GP].T  # [din, 4H]
                    for m in range(8):
                        c0 = (d * 8 + m) * P
                        wih_cols[0:din, c0:c0 + P] = Wt[:, m * P:(m + 1) * P].astype(NPBF)
                        wih_cols[din, c0:c0 + P] = bb_[d][GP][m * P:(m + 1) * P].astype(NPBF)
                put(name + "_wih", wih_cols)
                off[name + "_KT"] = 1
            else:
                KT = din // P
                wih_cols = np.zeros((P, 2 * KT * 8 * P), dtype=NPBF)
                for d in range(2):
                    Wt = Wih[d][GP].T  # [din, 4H]
                    for k in range(KT):
                        for m in range(8):
                            c0 = ((d * KT + k) * 8 + m) * P
                            wih_cols[:, c0:c0 + P] = Wt[k * P:(k + 1) * P, m * P:(m + 1) * P].astype(NPBF)
                put(name + "_wih", wih_cols)
                off[name + "_KT"] = KT
                bias_cols = np.zeros((1, 2 * 8 * P), dtype=NPBF)
                for d in range(2):
                    bias_cols[0, d * 8 * P:(d + 1) * 8 * P] = bb_[d][GP].astype(NPBF)
                put(name + "_bias", np.concatenate([bias_cols, np.zeros((P - 1, 2 * 8 * P), NPBF)], 0))

    # conv heads
    for hk, chain in HEADS.items():
        cps = params[hk]
        for ci, (Cout, Cin, kk) in enumerate(chain):
            w = _to_np(cps[ci]["w"]).astype(np.float32)  # [Cout, Cin, kk]
            cb = _to_np(cps[ci]["b"]).astype(np.float32)  # [Cout]
            KTin = (Cin + P - 1) // P
            MT = (Cout + P - 1) // P
            name = f"{hk}_{ci}"
            cols = []
            for tap in range(kk):
                Wt = w[:, :, tap].T  # [Cin, Cout]
                Wt_pad = np.zeros((KTin * P, Cout), np.float32)
                Wt_pad[:Cin] = Wt
                for k in range(KTin):
                    for m in range(MT):
                        mc = min(P, Cout - m * P)
                        t_ = np.zeros((P, mc), dtype=NPBF)
                        t_[:, :] = Wt_pad[k * P:(k + 1) * P, m * P:m * P + mc].astype(NPBF)
                        cols.append(t_)
            arr = np.concatenate(cols, axis=1)
            put(name + "_w", arr)
            off[name + "_wcols"] = arr.shape[1]
            if Cout == 1:
                off[name + "_bscalar"] = float(cb[0])
            bias_arr = np.zeros((P, MT), np.float32)
            for m in range(MT):
                mc = min(P, Cout - m * P)
                bias_arr[0:mc, m] = cb[m * P:m * P + mc]
            put_row0(name + "_b", bias_arr)

    WN = cur.n
    wpack = np.zeros((P, WN), dtype=NPBF)
    for o, arr in blocks:
        wpack[:, o:o + arr.shape[1]] = arr
    WN32 = max(cur32.n, 4)
    wpack32 = np.zeros((P, WN32), dtype=np.float32)
    for o, arr in blocks32:
        wpack32[0:arr.shape[0], o:o + arr.shape[1]] = arr
    return wpack, wpack32, off


# xin layout (bf16): bb [0:256], in5 [256:3328], h0 [3328:4352]
XOFF_BB, XOFF_IN5, XOFF_H0 = 0, 256, 3328
XN = 4352
XN32 = 1024  # c0: [128, (ld, (half,d,b))]

# (stage, layer) -> global layer-dir index base
LD_INDEX = {}
_ld = 0
for _st in STAGES:
    for _li in range(_st["nl"]):
        LD_INDEX[(_st["key"], _li)] = _ld
        _ld += 1


def _pack_core_inputs(x1, x3, eh, ec, core):
    s = slice(core * b, (core + 1) * b)
    xin = np.zeros((P, XN), dtype=NPBF)
    # bb: x1[:, ::12] -> [b, 8, 2]; rows 0-1 feats, row 2 ones
    bb_ = x1[s, ::12, :]  # [b, 8, 2]
    xin[0:2, XOFF_BB:XOFF_BB + 256] = bb_.transpose(2, 1, 0).reshape(2, 256).astype(NPBF)
    xin[2, XOFF_BB:XOFF_BB + 256] = 1.0
    # in5: concat(flip(x3[:, :, 0:1], t), x1) -> [b, 96, 3]; row 3 ones
    in5 = np.concatenate([x3[s, ::-1, 0:1], x1[s]], axis=2)  # [b, 96, 3]
    xin[0:3, XOFF_IN5:XOFF_IN5 + 3072] = in5.transpose(2, 1, 0).reshape(3, 3072).astype(NPBF)
    xin[3, XOFF_IN5:XOFF_IN5 + 3072] = 1.0
    # h0: [128, (ld, d, k, b)]
    for (sk, li), ld in LD_INDEX.items():
        st = next(t for t in STAGES if t["key"] == sk)
        for d in range(2):
            hv = eh[st["hidx"], 2 * li + d, s, :]  # [b, H]
            for k in range(2):
                c0 = ((ld * 2 + d) * 2 + k) * b
                xin[:, XOFF_H0 + c0:XOFF_H0 + c0 + b] = hv[:, k * P:(k + 1) * P].T.astype(NPBF)
    xin32 = np.zeros((P, XN32), dtype=np.float32)
    # c0 layout: [128, (ld, d, half, b)] so each direction's slice is contiguous
    for (sk, li), ld in LD_INDEX.items():
        st = next(t for t in STAGES if t["key"] == sk)
        for d in range(2):
            cv = ec[st["hidx"], 2 * li + d, s, :]  # [b, H]
            for k in range(2):
                c0 = ld * 128 + d * 64 + k * 32
                xin32[:, c0:c0 + b] = cv[:, k * P:(k + 1) * P].T
    return xin, xin32


def _mkap(base_ap, extra_off, dims):
    return bass.AP(base_ap.tensor, base_ap.offset + extra_off, [list(base_ap.ap[0])] + dims)


def _build_program(off):
    nc = bacc.Bacc(None, target_bir_lowering=False)
    WN = off["_WN"]
    WN32 = off["_WN32"]
    wpack = nc.dram_tensor("wpack", [P, WN], BF16, kind="ExternalInput")
    wpack32 = nc.dram_tensor("wpack32", [P, WN32], F32, kind="ExternalInput")
    xin = nc.dram_tensor("xin", [P, XN], BF16, kind="ExternalInput")
    xin32 = nc.dram_tensor("xin32", [P, XN32], F32, kind="ExternalInput")
    y = nc.dram_tensor("y", [1, YN], F32, kind="ExternalOutput")
    import os as _os
    dbg_mode = _os.environ.get("KERNEL_DEBUG_DUMP", "") == "1"
    dbg = {}
    if dbg_mode:
        for st in STAGES:
            dbg[st["key"]] = nc.dram_tensor(f"dbg_{st['key']}", [P, 4, (st["T"] + 4) * 32],
                                            BF16, kind="ExternalOutput")

    with tile.TileContext(nc) as tc:
        with tc.tile_pool(name="pers", bufs=1) as pers:
            bbT = pers.tile([P, 1, 256], BF16)
            nc.sync.dma_start(bbT[:], xin[:, XOFF_BB:XOFF_BB + 256].rearrange("p (o c) -> p o c", o=1))
            in5T = pers.tile([P, 1, 3072], BF16)
            nc.sync.dma_start(in5T[:], xin[:, XOFF_IN5:XOFF_IN5 + 3072].rearrange("p (o c) -> p o c", o=1))
            h0T = pers.tile([P, 1024], BF16)
            nc.sync.dma_start(h0T[:], xin[:, XOFF_H0:XOFF_H0 + 1024])
            c0T = pers.tile([P, 1024], F32)
            nc.sync.dma_start(c0T[:], xin32[:])
            ones = pers.tile([1, P], BF16)
            nc.vector.memset(ones[:], 1.0)
            o04 = pers.tile([1, 3072], F32)

            # stage output buffers (kept until head phase); pad 2*32 cols each side
            outs = {}
            for st in STAGES:
                t_ = pers.tile([P, 4, (st["T"] + 4) * 32], BF16, name=f"out_{st['key']}")
                outs[st["key"]] = t_
                nc.vector.memset(t_[:, :, 0:64], 0.0)
                nc.vector.memset(t_[:, :, (st["T"] + 2) * 32:], 0.0)

            with tc.tile_pool(name="work", bufs=1) as work, \
                 tc.tile_pool(name="psg", bufs=1, space="PSUM") as psg:

                def lstm_layer(skey, li, T, KT, xgetter, outbuf, out_off, out_free):
                    name = f"{skey}_l{li}"
                    ld = LD_INDEX[(skey, li)]
                    folded_bias = off[name + "_KT"] == 1
                    whh_sb = work.tile([P, 2, 2, 8, P], BF16, tag="whh", name=f"whh_{name}")
                    nc.sync.dma_start(whh_sb[:], wpack[:, off[name + "_whh"]:off[name + "_whh"] + 4096]
                                      .rearrange("p (d k m c) -> p d k m c", d=2, k=2, m=8))
                    wih_sb = work.tile([P, 2, 4, 8, P], BF16, tag="wih", name=f"wih_{name}")
                    for d in range(2):
                        src0 = off[name + "_wih"] + d * KT * 8 * P
                        nc.sync.dma_start(
                            wih_sb[:, d, 0:KT],
                            wpack[:, src0:src0 + KT * 8 * P].rearrange("p (k m c) -> p k m c", k=KT, m=8))
                    if not folded_bias:
                        bias_sb = work.tile([1, 2048], BF16, tag="bias", name=f"bias_{name}")
                        nc.sync.dma_start(bias_sb[:], wpack[0:1, off[name + "_bias"]:off[name + "_bias"] + 2048])
                    cbufs = [work.tile([P, 64], F32, tag=f"c{d}", name=f"c{d}_{name}") for d in range(2)]
                    NB = T // 4
                    for bi in range(NB):
                        gpsd = [psg.tile([P, 2, 512], F32, tag=f"gates{d}", bufs=2,
                                         name=f"g{d}_{name}_{bi}") for d in range(2)]
                        # blocked input projections (+bias) into PSUM
                        for d in range(2):
                            gps = gpsd[d]
                            seqbase = 4 * bi if d == 0 else T - 4 - 4 * bi
                            for m in range(8):
                                dst = gps[:, m >> 2, (m & 3) * 128:(m & 3) * 128 + 128]
                                first_of_bank = (m & 3) == 0
                                if not folded_bias:
                                    nc.tensor.matmul(dst, bias_sb[0:1, (d * 8 + m) * P:(d * 8 + m + 1) * P],
                                                     ones[0:1, 0:P],
                                                     start=first_of_bank, stop=False)
                                for k in range(KT):
                                    nc.tensor.matmul(dst, wih_sb[:, d, k, m, :],
                                                     xgetter(k, seqbase),
                                                     start=(folded_bias and first_of_bank and k == 0),
                                                     stop=False)
                        for j in range(4):
                            te = 4 * bi + j
                            for d in range(2):
                                gps = gpsd[d]
                                slot = j if d == 0 else 3 - j
                                seq = te if d == 0 else T - 1 - te
                                sp = te - 1 if d == 0 else T - te  # previous h position
                                for m in range(8):
                                    for k in range(2):
                                        if te == 0:
                                            rhs = h0T[:, ((ld * 2 + d) * 2 + k) * b:((ld * 2 + d) * 2 + k) * b + b]
                                        else:
                                            rhs = outbuf[:, d * 2 + k,
                                                         out_off + sp * 32:out_off + sp * 32 + 32]
                                        c0_ = (m & 3) * 128 + slot * 32
                                        nc.tensor.matmul(
                                            gps[:, m >> 2, c0_:c0_ + 32],
                                            whh_sb[:, d, k, m, :], rhs,
                                            start=False,
                                            stop=(j == 3 and k == 1 and (m & 3) == 3))
                                # activations straight from PSUM; per-direction chain
                                gact = work.tile([P, 256], F32, tag=f"gact{d}", bufs=3,
                                                 name=f"ga{d}_{name}_{te}")
                                nc.scalar.activation(
                                    _mkap(gact[:], 0, [[32, 6], [1, 32]]),
                                    _mkap(gps[:], slot * 32, [[128, 6], [1, 32]]),
                                    AF.Sigmoid)
                                nc.scalar.activation(
                                    _mkap(gact[:], 192, [[32, 2], [1, 32]]),
                                    _mkap(gps[:], 6 * 128 + slot * 32, [[128, 2], [1, 32]]),
                                    AF.Tanh)
                                cbuf = cbufs[d]
                                cprev = (c0T[:, ld * 128 + d * 64:ld * 128 + d * 64 + 64]
                                         if te == 0 else cbuf[:])
                                tmp1 = work.tile([P, 64], F32, tag=f"tmp1{d}", bufs=2,
                                                 name=f"t1{d}_{name}_{te}")
                                tmp2 = work.tile([P, 64], F32, tag=f"tmp2{d}", bufs=2,
                                                 name=f"t2{d}_{name}_{te}")
                                # i*g first: it has no dependence on the carried c,
                                # so DVE starts it before the c-chain arrives
                                nc.vector.tensor_mul(tmp2[:], gact[:, 0:64], gact[:, 192:256])
                                nc.vector.tensor_mul(tmp1[:], gact[:, 64:128], cprev)
                                nc.vector.tensor_add(cbuf[:], tmp1[:], tmp2[:])
                                tanc = work.tile([P, 64], F32, tag=f"tanc{d}", bufs=2,
                                                 name=f"tc{d}_{name}_{te}")
                                nc.scalar.activation(tanc[:], cbuf[:], AF.Tanh)
                                o_ap = _mkap(outbuf[:], (d * 2) * out_free + out_off + seq * 32,
                                             [[out_free, 2], [1, 32]])
                                nc.vector.tensor_mul(
                                    o_ap,
                                    gact[:, 128:192].rearrange("p (h c) -> p h c", h=2),
                                    tanc[:].rearrange("p (h c) -> p h c", h=2))

                def stage_build(st):
                    skey, T = st["key"], st["T"]
                    outb = outs[skey]
                    out_free = (T + 4) * 32
                    # input
                    if skey == "s0":
                        xsrc, x_kt = bbT, 1
                    elif skey == "s5":
                        xsrc, x_kt = in5T, 1
                    else:
                        nsrc, r = st["up"]
                        prev = STAGES[[s_["key"] for s_ in STAGES].index(skey) - 1]
                        pout = outs[prev["key"]]
                        p_free = (prev["T"] + 4) * 32
                        upin = work.tile([P, 4, 3072], BF16, tag="upin", name=f"up_{skey}")
                        srcv = pout[:, :, 64:64 + nsrc * 32].rearrange("p t (s c) -> p t s c", c=32)
                        for j in range(r):
                            nc.vector.tensor_copy(
                                _mkap(upin[:], j * 32, [[3072, 4], [r * 32, nsrc], [1, 32]]),
                                srcv)
                        xsrc, x_kt = upin, 4

                    for li in range(st["nl"]):
                        if li == 0:
                            KT = 1 if st["din0"] < P else 4
                            src = xsrc
                            def xget(k, seqbase, _s=src):
                                return _s[:, k, seqbase * 32:(seqbase + 4) * 32]
                        else:
                            KT = 4
                            l1 = l1o
                            def xget(k, seqbase, _s=l1):
                                return _s[:, k, seqbase * 32:(seqbase + 4) * 32]
                        last = li == st["nl"] - 1
                        if last:
                            obuf, ooff, ofree = outb, 64, out_free
                        else:
                            l1o = work.tile([P, 4, 3072], BF16, tag="l1o", name=f"l1o_{skey}")
                            obuf, ooff, ofree = l1o, 0, 3072
                        lstm_layer(skey, li, T, KT, xget, obuf, ooff, ofree)
                    if st["res"]:
                        for t_ in range(4):
                            nc.vector.tensor_add(outb[:, t_, 64:64 + T * 32],
                                                 outb[:, t_, 64:64 + T * 32],
                                                 xsrc[:, t_, 0:T * 32])

                for st in STAGES:
                    stage_build(st)
                    if dbg_mode:
                        nc.sync.dma_start(dbg[st["key"]][:], outs[st["key"]][:])

            # ---- head phase (all stage outputs alive) ----
            with tc.tile_pool(name="convw", bufs=2) as cwp, \
                 tc.tile_pool(name="hwork", bufs=1) as hw, \
                 tc.tile_pool(name="psc", bufs=1, space="PSUM") as psc:

                def head(st):
                    hk = st["head"]
                    chain = HEADS[hk]
                    T = st["T"]
                    src, src_off, src_kt = outs[st["key"]], 64, 4
                    src_rows = P
                    for ci, (Cout, Cin, kk) in enumerate(chain):
                        name = f"{hk}_{ci}"
                        KTin = (Cin + P - 1) // P
                        MT = (Cout + P - 1) // P
                        pad = kk // 2
                        wcols = off[name + "_wcols"]
                        wtile = cwp.tile([P, wcols], BF16, tag="cw", name=f"cw_{name}")
                        nc.sync.dma_start(wtile[:], wpack[:, off[name + "_w"]:off[name + "_w"] + wcols])
                        btile = cwp.tile([P, MT], F32, tag="cb", name=f"cb_{name}")
                        nc.sync.dma_start(btile[:], wpack32[:, off[name + "_b"]:off[name + "_b"] + MT])
                        last = ci == len(chain) - 1
                        if not last:
                            padn = chain[ci + 1][2] // 2
                            mid = hw.tile([P, MT, (T + 2 * padn) * 32], BF16,
                                          tag=f"mid{ci}", name=f"mid_{name}")
                            nc.vector.memset(mid[:, :, 0:padn * 32], 0.0)
                            nc.vector.memset(mid[:, :, (T + padn) * 32:], 0.0)
                            if Cout < P:
                                # rows above Cout would otherwise hold stale data;
                                # next conv contracts K=128 against zero-padded weights
                                nc.vector.memset(mid[Cout:P, :, :], 0.0)
                        TB = T * 32
                        for m in range(MT):
                            mc = min(P, Cout - m * P)
                            for ch0 in range(0, TB, 512):
                                n = min(512, TB - ch0)
                                ps = psc.tile([P, 512], F32, tag="cps", bufs=4, name=f"ps_{name}_{m}_{ch0}")
                                cnt = 0
                                for tap_i in range(kk):
                                    for k in range(KTin):
                                        col = (tap_i * KTin + k) * Cout + m * P
                                        rhs = src[:, k, src_off + ch0 + (tap_i - pad) * 32:
                                                  src_off + ch0 + (tap_i - pad) * 32 + n]
                                        nc.tensor.matmul(ps[0:mc, 0:n], wtile[:, col:col + mc], rhs,
                                                         start=(cnt == 0),
                                                         stop=(cnt == kk * KTin - 1))
                                        cnt += 1
                                if not last:
                                    nc.scalar.activation(mid[0:mc, m, padn * 32 + ch0:padn * 32 + ch0 + n],
                                                         ps[0:mc, 0:n], AF.Tanh,
                                                         bias=btile[0:mc, m:m + 1])
                                else:
                                    hsb = hw.tile([1, 512], F32, tag="hchunk", bufs=3, name=f"h_{name}_{ch0}")
                                    nc.scalar.activation(hsb[0:1, 0:n], ps[0:1, 0:n], AF.Copy,
                                                         bias=off[name + "_bscalar"])
                                    if st["key"] == "s4":
                                        nc.vector.tensor_copy(o04[0:1, ch0:ch0 + n], hsb[0:1, 0:n])
                                    elif st["key"] == "s5":
                                        nc.sync.dma_start(y[0:1, YSEG["out2"][0] + ch0:
                                                            YSEG["out2"][0] + ch0 + n], hsb[0:1, 0:n])
                                        o4c = hw.tile([1, 512], F32, tag="o4c", bufs=3, name=f"o4_{ch0}")
                                        nc.vector.tensor_add(o4c[0:1, 0:n], hsb[0:1, 0:n],
                                                             o04[0:1, ch0:ch0 + n])
                                        nc.sync.dma_start(y[0:1, YSEG["out4"][0] + ch0:
                                                            YSEG["out4"][0] + ch0 + n], o4c[0:1, 0:n])
                                    else:
                                        seg = {"s0": "out0", "s1": "out1", "s3": "out3"}[st["key"]]
                                        nc.sync.dma_start(y[0:1, YSEG[seg][0] + ch0:
                                                            YSEG[seg][0] + ch0 + n], hsb[0:1, 0:n])
                        if not last:
                            src, src_off, src_kt = mid, padn * 32, MT

                # s4 before s5 so o04 is ready for out4
                for st in [STAGES[0], STAGES[1], STAGES[2], STAGES[3], STAGES[4]]:
                    head(st)

    nc.finalize()
    return nc


_CACHE = {}


def kernel(x1, x3, encoder_h, encoder_c, params):
    x1 = _to_np(x1).astype(np.float32)
    x3 = _to_np(x3).astype(np.float32)
    eh = _to_np(encoder_h).astype(np.float32)
    ec = _to_np(encoder_c).astype(np.float32)

    if "prog" not in _CACHE:
        wpack, wpack32, off = _pack_weights(params)
        off["_WN"] = wpack.shape[1]
        off["_WN32"] = wpack32.shape[1]
        nc = _build_program(off)
        _CACHE["prog"] = (nc, wpack, wpack32)
    nc, wpack, wpack32 = _CACHE["prog"]

    in_maps = []
    for core in range(NCORES):
        xin, xin32 = _pack_core_inputs(x1, x3, eh, ec, core)
        in_maps.append({"wpack": wpack, "wpack32": wpack32, "xin": xin, "xin32": xin32})
    res = run_bass_kernel_spmd(nc, in_maps, core_ids=list(range(NCORES)))
    _CACHE["last_result"] = res

    y_all = np.stack([res.results[c]["y"][0] for c in range(NCORES)])  # [8, YN]

    def seg(name, T):
        o, ln = YSEG[name]
        assert ln == T * b
        return np.concatenate([y_all[c, o:o + ln].reshape(T, b).T for c in range(NCORES)], axis=0)

    out0 = seg("out0", 8)
    out1 = seg("out1", 12)
    out2 = seg("out2", 96)
    out3 = seg("out3", 36)
    out4 = seg("out4", 96)
    return (out0.astype(np.float32), out1.astype(np.float32), out2.astype(np.float32),
            out3.astype(np.float32), out4.astype(np.float32))
